# revision 5
# baseline (speedup 1.0000x reference)
"""DiffHead (differential attention, single head) Trainium2 kernel.

Sharding: 8 cores = 4 batches x 2 softmax components. Each core computes one
full causal attention (softmax(Qc Kc^T * scale) @ V) for one batch and one
component c in {1,2}; the host combines out_b = O1_b - lambda * O2_b.

Per-core layouts (host-marshaled):
  qT,kT,vT : [C=1024, T=2048] bf16  (pre-transposed so the contraction dim C
                                     lands on SBUF partitions with fast DMA)
  wq,wk,wv : [C=1024, H=128]  bf16  (component slice of the projection weight)
  out      : [T=2048, HO=128] f32   (normalized single-component attention out)
"""

import numpy as np
import ml_dtypes
from contextlib import ExitStack

import concourse.bass as bass
import concourse.mybir as mybir
import concourse.tile as tile
from concourse import bacc
from concourse import bass_utils
from concourse.masks import make_identity

T, C, H, HO = 2048, 1024, 128, 128
SCALE = float(H) ** -0.5
LAMBDA_INIT = 0.8
TQ = 512            # q-tile width for S^T tiles (PSUM bank = 512 f32)
NCC = C // 128      # 8 contraction chunks
NKC = T // 128      # 16 key chunks
NQT = T // TQ       # 4 q tiles
BF16 = mybir.dt.bfloat16
F32 = mybir.dt.float32
EXP = mybir.ActivationFunctionType.Exp


def _emit_kernel(ctx: ExitStack, tc, qT, kT, vT, wq, wk, wv, out):
    nc = tc.nc
    consts = ctx.enter_context(tc.tile_pool(name="consts", bufs=1))
    wpool = ctx.enter_context(tc.tile_pool(name="wpool", bufs=1))
    inpool = ctx.enter_context(tc.tile_pool(name="inpool", bufs=16))
    actpool = ctx.enter_context(tc.tile_pool(name="actpool", bufs=1))
    vppool = ctx.enter_context(tc.tile_pool(name="vppool", bufs=1))
    ptpool = ctx.enter_context(tc.tile_pool(name="ptpool", bufs=2))
    outpool = ctx.enter_context(tc.tile_pool(name="outpool", bufs=4))
    ps_proj = ctx.enter_context(tc.tile_pool(name="ps_proj", bufs=2, space="PSUM"))
    ps_tr = ctx.enter_context(tc.tile_pool(name="ps_tr", bufs=2, space="PSUM"))
    ps_s = ctx.enter_context(tc.tile_pool(name="ps_s", bufs=2, space="PSUM"))
    ps_o = ctx.enter_context(tc.tile_pool(name="ps_o", bufs=2, space="PSUM"))

    identity = consts.tile([128, 128], BF16)
    make_identity(nc, identity)

    w_sb = {}
    for name, w in (("wq", wq), ("wk", wk), ("wv", wv)):
        t_ = wpool.tile([128, NCC, H], BF16, tag=name)
        nc.sync.dma_start(out=t_, in_=w.rearrange("(n p) h -> p n h", p=128))
        w_sb[name] = t_

    # ---- Projections: Q^T, K^T, V^T, all [h=128 partitions, T free] bf16 ----
    QT = actpool.tile([128, T], BF16, tag="QT")
    KT = actpool.tile([128, T], BF16, tag="KT")
    VTt = actpool.tile([128, T], BF16, tag="VTt")
    for src, wname, dst in ((qT, "wq", QT), (kT, "wk", KT), (vT, "wv", VTt)):
        chunks = []
        for cc in range(NCC):
            ch = inpool.tile([128, T], BF16, tag="inchunk")
            nc.sync.dma_start(out=ch, in_=src[cc * 128:(cc + 1) * 128, :])
            chunks.append(ch)
        for tq in range(NQT):
            ps = ps_proj.tile([128, TQ], F32, tag="proj")
            for cc in range(NCC):
                nc.tensor.matmul(
                    ps,
                    lhsT=w_sb[wname][:, cc],
                    rhs=chunks[cc][:, tq * TQ:(tq + 1) * TQ],
                    start=(cc == 0),
                    stop=(cc == NCC - 1),
                )
            nc.vector.tensor_copy(out=dst[:, tq * TQ:(tq + 1) * TQ], in_=ps)

    # ---- V' = [V | ones] with T_k on partitions: [128, NKC, HO+1] bf16 ----
    Vp = vppool.tile([128, NKC, HO + 1], BF16, tag="vp")
    nc.vector.memset(Vp[:, :, HO:HO + 1], 1.0)
    for j in range(NKC):
        pst = ps_tr.tile([128, 128], BF16, tag="tr")
        nc.tensor.transpose(pst, VTt[:, j * 128:(j + 1) * 128], identity)
        nc.vector.tensor_copy(out=Vp[:, j, 0:HO], in_=pst)

    # ---- Attention per q-tile ----
    for i in range(NQT):
        nj = min(4 * i + 5, NKC)
        # P^T tiles: [T_k chunk on partitions, T_q free], bf16, causal-masked
        PT = ptpool.tile([128, NKC, TQ], BF16, tag="pt")
        for j in range(nj):
            pss = ps_s.tile([128, TQ], F32, tag="s")
            nc.tensor.matmul(
                pss,
                lhsT=KT[:, j * 128:(j + 1) * 128],
                rhs=QT[:, i * TQ:(i + 1) * TQ],
                start=True,
                stop=True,
            )
            # P^T = exp(S^T * scale); logits are O(1) so no max-subtraction
            nc.scalar.activation(out=PT[:, j], in_=pss, func=EXP, scale=SCALE)
            if j >= 4 * i:
                # causal tril(diagonal=1): keep iff (512i+f)+1-(128j+p) >= 0
                nc.gpsimd.affine_select(
                    out=PT[:, j],
                    in_=PT[:, j],
                    compare_op=mybir.AluOpType.is_ge,
                    fill=0.0,
                    base=TQ * i - 128 * j + 1,
                    channel_multiplier=-1,
                    pattern=[[1, TQ]],
                )
        for mi in range(4):
            m = 4 * i + mi
            jmax = min(m + 1, NKC - 1)
            pso = ps_o.tile([128, HO + 1], F32, tag="o")
            for j in range(jmax + 1):
                nc.tensor.matmul(
                    pso,
                    lhsT=PT[:, j, mi * 128:(mi + 1) * 128],
                    rhs=Vp[:, j],
                    start=(j == 0),
                    stop=(j == jmax),
                )
            rec = outpool.tile([128, 1], F32, tag="rec")
            nc.vector.reciprocal(rec, pso[:, HO:HO + 1])
            osb = outpool.tile([128, HO], F32, tag="osb")
            nc.vector.tensor_scalar_mul(osb, pso[:, 0:HO], rec)
            nc.sync.dma_start(out=out[m * 128:(m + 1) * 128, :], in_=osb)


def build_nc():
    nc = bacc.Bacc("TRN2", target_bir_lowering=False, debug=False)
    aps = {}
    for name in ("qT", "kT", "vT"):
        aps[name] = nc.dram_tensor(name, [C, T], BF16, kind="ExternalInput").ap()
    for name in ("wq", "wk", "wv"):
        aps[name] = nc.dram_tensor(name, [C, H], BF16, kind="ExternalInput").ap()
    out = nc.dram_tensor("out", [T, HO], F32, kind="ExternalOutput").ap()
    with tile.TileContext(nc) as tc:
        with ExitStack() as ctx:
            _emit_kernel(ctx, tc, aps["qT"], aps["kT"], aps["vT"],
                         aps["wq"], aps["wk"], aps["wv"], out)
    nc.compile()
    return nc


def make_in_maps(q, k, v, Wq, Wk, Wv):
    bf16 = ml_dtypes.bfloat16
    B = q.shape[0]
    in_maps = []
    for b in range(B):
        qT = np.ascontiguousarray(q[b].T).astype(bf16)
        kT = np.ascontiguousarray(k[b].T).astype(bf16)
        vT = np.ascontiguousarray(v[b].T).astype(bf16)
        for c in range(2):
            in_maps.append({
                "qT": qT, "kT": kT, "vT": vT,
                "wq": np.ascontiguousarray(Wq[:, c * H:(c + 1) * H]).astype(bf16),
                "wk": np.ascontiguousarray(Wk[:, c * H:(c + 1) * H]).astype(bf16),
                "wv": np.ascontiguousarray(Wv).astype(bf16),
            })
    return in_maps


def kernel_impl(q, k, v, Wq, Wk, Wv, lambda_q1, lambda_k1, lambda_q2, lambda_k2,
                trace=False):
    B = q.shape[0]
    lbd = (np.exp(np.dot(lambda_q1.astype(np.float32), lambda_k1.astype(np.float32)))
           - np.exp(np.dot(lambda_q2.astype(np.float32), lambda_k2.astype(np.float32)))
           + np.float32(LAMBDA_INIT))
    in_maps = make_in_maps(q, k, v, Wq, Wk, Wv)
    nc = build_nc()
    res = bass_utils.run_bass_kernel_spmd(
        nc, in_maps, core_ids=list(range(len(in_maps))), trace=trace)
    outs = [res.results[i]["out"] for i in range(len(in_maps))]
    full = np.stack([outs[2 * b] - lbd * outs[2 * b + 1] for b in range(B)])
    return full.astype(np.float32), res


def kernel(q, k, v, Wq, Wk, Wv, lambda_q1, lambda_k1, lambda_q2, lambda_k2):
    out, _ = kernel_impl(q, k, v, Wq, Wk, Wv,
                         lambda_q1, lambda_k1, lambda_q2, lambda_k2)
    return out
